# revision 7
# baseline (speedup 1.0000x reference)
"""CrossAttention Trainium2 Bass kernel (v3: SBUF-resident intermediates,
row-packed scores, permuted head ordering, interleaved projections).

Problem (hardcoded shapes): B=8, N=S=1024, DIM=1024, H=16, DH=64.
  q = (queries @ Wq.T).reshape(B, H, N, DH)   # direct reshape, NOT a head transpose
  attn = softmax(q @ k^T * DH**-0.5); out = attn @ v
  out = out.transpose(0,2,1,3).reshape(B,N,H*DH) @ Wo.T + bo
Sharding: data-parallel over batch B (one batch element per core, weights
replicated, no collectives).

Key ideas vs v2 (391us baseline):
- QnT/KnT/Vn stay SBUF-resident (split into js/t halves as separate tiles so
  heads 0-7 only depend on the first half of the projections); head operand
  gathers are SBUF->SBUF DMA, no DRAM bounce.
- Heads use the permuted local ordering n~ = g*64+r (instead of the torch
  n' = r*16+g): the gather then lands directly in matmul layout (no DVE
  reorder); softmax is permutation-equivariant, and the inverse permutation
  is applied for free in the divide's strided write to outT.
- Scores for a head pair run as concurrent 64-row PE tiles ((0,0)/(64,0)):
  contraction is DH=64, so two heads share the 128x128 array -> 2x scores.
- The second half of the projections (q/k js=1, v t=4..7) is woven between
  the first 4 head-pairs' chunks, sharing one PSUM tag, so the PE stays
  dense (HAM warm) while ScalarE exp is the per-pair bottleneck.
- Softmax divide: DVE reciprocal straight from the PSUM Z row, one broadcast
  DMA, one tensor_mul (strided dst applies the n~ -> n' permutation).
"""

import numpy as np

import concourse.bass as bass
import concourse.mybir as mybir
import concourse.tile as tile
from concourse import bacc

B, N, S, DIM, H, DH = 8, 1024, 1024, 1024, 16, 64
SCALE = DH**-0.5
P = 128
F32 = mybir.dt.float32
BF16 = mybir.dt.bfloat16
AF = mybir.ActivationFunctionType


def build(debug: bool = False) -> bacc.Bacc:
    nc = bacc.Bacc("TRN2", target_bir_lowering=False, debug=debug, num_devices=B)

    xqT = nc.dram_tensor("xqT", [DIM, N], BF16, kind="ExternalInput")
    xkT = nc.dram_tensor("xkT", [DIM, S], BF16, kind="ExternalInput")
    xvT = nc.dram_tensor("xvT", [DIM, S], BF16, kind="ExternalInput")
    wqT = nc.dram_tensor("wqT", [DIM, H * DH], BF16, kind="ExternalInput")
    wkT = nc.dram_tensor("wkT", [DIM, H * DH], BF16, kind="ExternalInput")
    wvT = nc.dram_tensor("wvT", [DIM, H * DH], BF16, kind="ExternalInput")
    woT = nc.dram_tensor("woT", [H * DH, DIM], BF16, kind="ExternalInput")
    bo = nc.dram_tensor("bo", [1, DIM], F32, kind="ExternalInput")
    out = nc.dram_tensor("out", [N, DIM], F32, kind="ExternalOutput")

    with tile.TileContext(nc) as tc:
        with (
            tc.tile_pool(name="const", bufs=1) as const,
            tc.tile_pool(name="persist", bufs=1) as persist,
            tc.tile_pool(name="wtail", bufs=1) as wtail,
            tc.tile_pool(name="heads", bufs=2) as heads,
            tc.tile_pool(name="v66p", bufs=4) as v66p,
            tc.tile_pool(name="work", bufs=3) as work,
            tc.tile_pool(name="sm", bufs=2) as sm,
            tc.tile_pool(name="mm_psum", bufs=2, space="PSUM") as mm_psum,
            tc.tile_pool(name="o_psum", bufs=2, space="PSUM") as o_psum,
            tc.tile_pool(name="dram", bufs=1, space="DRAM") as dram,
        ):
            ones8 = const.tile([P, 8, 1], BF16)
            nc.vector.memset(ones8, 1.0)
            bo_bc = persist.tile([P, DIM], F32)
            nc.sync.dma_start(bo_bc, bo[:].to_broadcast((P, DIM)))

            # SBUF-resident operands, [c, ck, free] with c = ck*128 + p
            xq_sb = persist.tile([P, 8, N], BF16, name="xq_sb")
            xk_sb = persist.tile([P, 8, S], BF16, name="xk_sb")
            xv_sb = persist.tile([P, 8, S], BF16, name="xv_sb")
            wk_sb = persist.tile([P, 8, H * DH], BF16, name="wk_sb")
            wv_sb = persist.tile([P, 8, H * DH], BF16, name="wv_sb")
            # wq's slot is reused by wo (wq is dead once q js=1 is emitted)
            wq_sb = wtail.tile([P, 8, H * DH], BF16, tag="wt", name="wq_sb")

            def load_in(src, dst):
                v = src[:].rearrange("(ck c) f -> c ck f", c=P)
                for i in range(4):
                    nc.sync.dma_start(dst[:, 2 * i : 2 * i + 2, :], v[:, 2 * i : 2 * i + 2, :])

            # load order: q-projection operands first so stage A starts early
            load_in(wqT, wq_sb)
            load_in(xqT, xq_sb)
            load_in(wkT, wk_sb)
            load_in(xkT, xk_sb)
            load_in(wvT, wv_sb)
            load_in(xvT, xv_sb)

            # projections, SBUF-resident: QnT/KnT [j = t*128+p, n] split by n
            # half; Vn [n = t*128+p, j] split by n half (t 0-3 / 4-7)
            QnT_h = [persist.tile([P, 8, 512], BF16, name=f"QnT_{i}") for i in range(2)]
            KnT_h = [persist.tile([P, 8, 512], BF16, name=f"KnT_{i}") for i in range(2)]
            Vn_h = [persist.tile([P, 4, 1024], BF16, name=f"Vn_{i}") for i in range(2)]
            outT = persist.tile([P, 8, N], BF16, name="outT")

            def gemm_group(lhs_sb, rhs_sb, t, js, dst_ap, tag):
                """one output tile [128, 512]: 8 accumulating MMs + cast"""
                pp = mm_psum.tile([P, 512], F32, tag="mm", name=f"pp_{tag}_{t}_{js}")
                for ck in range(8):
                    nc.tensor.matmul(
                        pp,
                        lhs_sb[:, ck, t * P : (t + 1) * P],
                        rhs_sb[:, ck, js * 512 : (js + 1) * 512],
                        start=(ck == 0),
                        stop=(ck == 7),
                    )
                nc.vector.tensor_copy(dst_ap, pp)

            def q_group(t, js):
                gemm_group(wq_sb, xq_sb, t, js, QnT_h[js][:, t, :], "q")

            def k_group(t, js):
                gemm_group(wk_sb, xk_sb, t, js, KnT_h[js][:, t, :], "k")

            def v_group(t, js):
                gemm_group(xv_sb, wv_sb, t, js, Vn_h[t // 4][:, t % 4, js * 512 : (js + 1) * 512], "v")

            # ---- stage A first half: everything heads 0-7 need ----
            for t in range(8):
                q_group(t, 0)
            for t in range(8):
                k_group(t, 0)
            for t in range(4):
                v_group(t, 0)
                v_group(t, 1)

            # remaining projection groups, woven into the first head pairs
            weave = (
                [("q", t, 1) for t in range(8)]
                + [("k", t, 1) for t in range(8)]
                + [("v", t, js) for t in range(4, 8) for js in range(2)]
            )
            weave_fns = {"q": q_group, "k": k_group, "v": v_group}
            weave_i = 0

            def weave_one():
                nonlocal weave_i
                if weave_i < len(weave):
                    kind, t, js = weave[weave_i]
                    weave_fns[kind](t, js)
                    weave_i += 1

            # wo loads into wq's slot; emitted after the weave list is built
            # but its DMA only runs once q js=1 has consumed wq.
            def emit_wo_load():
                wo_sb = wtail.tile([P, 8, DIM], BF16, tag="wt", name="wo_sb")
                load_in(woT, wo_sb)
                return wo_sb

            # ---- head pairs ----
            def pair(u, weave_per_chunk):
                h0, h1 = 2 * u, 2 * u + 1
                half = u // 4  # which n-half of QnT/KnT (heads 0-7 vs 8-15)
                c0 = h0 * 64 - half * 512
                c1 = h1 * 64 - half * 512

                # kT2/qT2: [128, 1024] bf16; partitions 0-63 = head h0's
                # [d, n~], 64-127 = head h1's. n~ = g*64+r (permuted order).
                kT2 = heads.tile([P, 1024], BF16, tag="kT2", name=f"kT2_{u}")
                qT2 = heads.tile([P, 1024], BF16, tag="qT2", name=f"qT2_{u}")
                for dst, src_t, cc in (
                    (kT2[0:64, :], KnT_h[half], c0),
                    (kT2[64:128, :], KnT_h[half], c1),
                    (qT2[0:64, :], QnT_h[half], c0),
                    (qT2[64:128, :], QnT_h[half], c1),
                ):
                    # dst logical [d, n~ = (2t+gp)*64 + r]; one DMA per gp
                    dv = dst.rearrange("d (t gp r) -> d t gp r", gp=2, r=64)
                    for gp in range(2):
                        nc.sync.dma_start(
                            dv[:, :, gp, :],
                            src_t[gp * 64 : (gp + 1) * 64, :, cc : cc + 64],
                        )

                # v66[p, sk, 0:64] = v~[sk*128+p, d]; col 64 = ones (rowsum)
                def v_gather(h):
                    v66 = v66p.tile([P, 8, 65], BF16, tag="v66", name=f"v66_{h}")
                    t = h // 2
                    r0 = (h % 2) * 64
                    src = Vn_h[t // 4][r0 : r0 + 64, t % 4, :].rearrange(
                        "r (s gp d) -> r s gp d", gp=2, d=DH
                    )
                    for gp in range(2):
                        nc.sync.dma_start(
                            v66[gp * 64 : (gp + 1) * 64, :, 0:64], src[:, :, gp, :]
                        )
                    nc.vector.tensor_copy(v66[:, :, 64:65], ones8)
                    return v66

                v66_0 = v_gather(h0)
                v66_1 = v_gather(h1)

                po_0 = o_psum.tile([P, 1024], F32, tag="po", name=f"po_{h0}")
                po_1 = o_psum.tile([P, 1024], F32, tag="po", name=f"po_{h1}")

                for sk in range(8):
                    ps_0 = mm_psum.tile([P, 1024], F32, tag="mm", name=f"ps_{h0}_{sk}")
                    ps_1 = mm_psum.tile([P, 1024], F32, tag="mm", name=f"ps_{h1}_{sk}")
                    # row-packed scores: head h0 on array rows 0-63, h1 on 64-127
                    for ns in range(2):
                        nc.tensor.matmul(
                            ps_0[:, ns * 512 : (ns + 1) * 512],
                            kT2[0:64, P * sk : P * (sk + 1)],
                            qT2[0:64, ns * 512 : (ns + 1) * 512],
                            start=True,
                            stop=True,
                        )
                        nc.tensor.matmul(
                            ps_1[:, ns * 512 : (ns + 1) * 512],
                            kT2[64:128, P * sk : P * (sk + 1)],
                            qT2[64:128, ns * 512 : (ns + 1) * 512],
                            start=True,
                            stop=True,
                        )
                    pexp_0 = work.tile([P, 1024], BF16, tag="pexp", name="pexp0")
                    nc.scalar.activation(pexp_0, ps_0, AF.Exp, scale=SCALE)
                    pexp_1 = work.tile([P, 1024], BF16, tag="pexp", name="pexp1")
                    nc.scalar.activation(pexp_1, ps_1, AF.Exp, scale=SCALE)
                    for po, v66, pexp in ((po_0, v66_0, pexp_0), (po_1, v66_1, pexp_1)):
                        for ns in range(2):
                            nc.tensor.matmul(
                                po[0:65, ns * 512 : (ns + 1) * 512],
                                v66[:, sk, 0:65],
                                pexp[:, ns * 512 : (ns + 1) * 512],
                                start=(sk == 0),
                                stop=(sk == 7),
                            )
                    for _ in range(weave_per_chunk):
                        weave_one()

                # softmax divide; Z on psum partition 64. The strided dst
                # applies n~ = g*64+r  ->  n' = r*16+g.
                jk = u
                for hi, po in ((0, po_0), (1, po_1)):
                    zr = sm.tile([1, 1024], BF16, tag="zr", name=f"zr_{u}_{hi}")
                    with nc.allow_low_precision(reason="softmax denom reciprocal in bf16"):
                        nc.vector.reciprocal(zr, po[64:65, :])
                    zrd = dram.tile([1, 1024], BF16, tag="zrd", name=f"zrd_{u}_{hi}")
                    nc.sync.dma_start(zrd, zr)
                    rbc = sm.tile([64, 1024], BF16, tag="rbc", name=f"rbc_{u}_{hi}")
                    nc.sync.dma_start(rbc, zrd[:].to_broadcast((64, 1024)))
                    in0 = po[0:64, :].rearrange("d (g r) -> d g r", g=16)
                    in1 = rbc.rearrange("d (g r) -> d g r", g=16)
                    if hi == 0:
                        nc.vector.tensor_mul(
                            out=outT[0:64, jk, :].rearrange("d (r g) -> d g r", g=16),
                            in0=in0,
                            in1=in1,
                        )
                    else:
                        tmpo = sm.tile([64, 1024], BF16, tag="tmpo", name=f"tmpo_{u}")
                        nc.vector.tensor_mul(
                            out=tmpo.rearrange("d (r g) -> d g r", g=16), in0=in0, in1=in1
                        )
                        nc.sync.dma_start(outT[64:128, jk, :], tmpo)

            for u in range(4):
                pair(u, weave_per_chunk=1)
            wo_sb = emit_wo_load()
            for u in range(4, 8):
                pair(u, weave_per_chunk=0)

            # ---- stage C: out = outT.T @ woT + bo ----
            for m in range(8):
                for isl in range(2):
                    pf = mm_psum.tile([P, 512], F32, tag="mm", name=f"pf_{m}_{isl}")
                    for ck in range(8):
                        nc.tensor.matmul(
                            pf,
                            outT[:, ck, m * P : (m + 1) * P],
                            wo_sb[:, ck, isl * 512 : (isl + 1) * 512],
                            start=(ck == 0),
                            stop=(ck == 7),
                        )
                    fin = sm.tile([P, 512], F32, tag="fin", name=f"fin_{m}_{isl}")
                    nc.vector.tensor_add(out=fin, in0=pf, in1=bo_bc[:, isl * 512 : (isl + 1) * 512])
                    nc.sync.dma_start(out[m * P : (m + 1) * P, isl * 512 : (isl + 1) * 512], fin)

    nc.compile()
    return nc


_NC_CACHE = {}


def _get_nc():
    if "nc" not in _NC_CACHE:
        _NC_CACHE["nc"] = build()
    return _NC_CACHE["nc"]


TRACE = False


def kernel(queries, keys, values, Wq, Wk, Wv, Wo, bo):
    import ml_dtypes
    from concourse.bass_utils import run_bass_kernel_spmd

    bf16 = ml_dtypes.bfloat16
    qT = [np.ascontiguousarray(np.asarray(queries[i], np.float32).T).astype(bf16) for i in range(B)]
    kT = [np.ascontiguousarray(np.asarray(keys[i], np.float32).T).astype(bf16) for i in range(B)]
    vT = [np.ascontiguousarray(np.asarray(values[i], np.float32).T).astype(bf16) for i in range(B)]
    wqT = np.ascontiguousarray(np.asarray(Wq, np.float32).T).astype(bf16)
    wkT = np.ascontiguousarray(np.asarray(Wk, np.float32).T).astype(bf16)
    wvT = np.ascontiguousarray(np.asarray(Wv, np.float32).T).astype(bf16)
    woT = np.ascontiguousarray(np.asarray(Wo, np.float32).T).astype(bf16)
    bo2 = np.ascontiguousarray(np.asarray(bo, np.float32).reshape(1, DIM))

    nc = _get_nc()
    in_maps = [
        {
            "xqT": qT[i],
            "xkT": kT[i],
            "xvT": vT[i],
            "wqT": wqT,
            "wkT": wkT,
            "wvT": wvT,
            "woT": woT,
            "bo": bo2,
        }
        for i in range(B)
    ]
    res = run_bass_kernel_spmd(nc, in_maps, core_ids=list(range(B)), trace=TRACE)
    if TRACE:
        _NC_CACHE["last_results"] = res
    return np.stack([res.results[i]["out"] for i in range(B)])


# revision 13
# speedup vs baseline: 1.3067x; 1.3067x over previous
"""CrossAttention Trainium2 Bass kernel (v3: SBUF-resident intermediates,
row-packed scores, permuted head ordering, interleaved projections).

Problem (hardcoded shapes): B=8, N=S=1024, DIM=1024, H=16, DH=64.
  q = (queries @ Wq.T).reshape(B, H, N, DH)   # direct reshape, NOT a head transpose
  attn = softmax(q @ k^T * DH**-0.5); out = attn @ v
  out = out.transpose(0,2,1,3).reshape(B,N,H*DH) @ Wo.T + bo
Sharding: data-parallel over batch B (one batch element per core, weights
replicated, no collectives).

Key ideas vs v2 (391us baseline):
- QnT/KnT/Vn stay SBUF-resident (split into js/t halves as separate tiles so
  heads 0-7 only depend on the first half of the projections); head operand
  gathers are SBUF->SBUF DMA, no DRAM bounce.
- Heads use the permuted local ordering n~ = g*64+r (instead of the torch
  n' = r*16+g): the gather then lands directly in matmul layout (no DVE
  reorder); softmax is permutation-equivariant, and the inverse permutation
  is applied for free in the divide's strided write to outT.
- Scores for a head pair run as concurrent 64-row PE tiles ((0,0)/(64,0)):
  contraction is DH=64, so two heads share the 128x128 array -> 2x scores.
- The second half of the projections (q/k js=1, v t=4..7) is woven between
  the first 4 head-pairs' chunks, sharing one PSUM tag, so the PE stays
  dense (HAM warm) while ScalarE exp is the per-pair bottleneck.
- Softmax divide: DVE reciprocal straight from the PSUM Z row, one broadcast
  DMA, one tensor_mul (strided dst applies the n~ -> n' permutation).
"""

import numpy as np

import concourse.bass as bass
import concourse.mybir as mybir
import concourse.tile as tile
from concourse import bacc

B, N, S, DIM, H, DH = 8, 1024, 1024, 1024, 16, 64
SCALE = DH**-0.5
P = 128
F32 = mybir.dt.float32
BF16 = mybir.dt.bfloat16
AF = mybir.ActivationFunctionType


def build(debug: bool = False) -> bacc.Bacc:
    nc = bacc.Bacc("TRN2", target_bir_lowering=False, debug=debug, num_devices=B)

    xqT = nc.dram_tensor("xqT", [DIM, N], BF16, kind="ExternalInput")
    xkT = nc.dram_tensor("xkT", [DIM, S], BF16, kind="ExternalInput")
    xvT = nc.dram_tensor("xvT", [DIM, S], BF16, kind="ExternalInput")
    wqT = nc.dram_tensor("wqT", [DIM, H * DH], BF16, kind="ExternalInput")
    wkT = nc.dram_tensor("wkT", [DIM, H * DH], BF16, kind="ExternalInput")
    wvT = nc.dram_tensor("wvT", [DIM, H * DH], BF16, kind="ExternalInput")
    woT = nc.dram_tensor("woT", [H * DH, DIM], BF16, kind="ExternalInput")
    bo = nc.dram_tensor("bo", [1, DIM], F32, kind="ExternalInput")
    out = nc.dram_tensor("out", [N, DIM], F32, kind="ExternalOutput")

    with tile.TileContext(nc) as tc:
        with (
            tc.tile_pool(name="const", bufs=1) as const,
            tc.tile_pool(name="persist", bufs=1) as persist,
            tc.tile_pool(name="wtail", bufs=1) as wtail,
            tc.tile_pool(name="heads", bufs=2) as heads,
            tc.tile_pool(name="v66p", bufs=4) as v66p,
            tc.tile_pool(name="work", bufs=3) as work,
            tc.tile_pool(name="sm", bufs=2) as sm,
            tc.tile_pool(name="mm_psum", bufs=2, space="PSUM") as mm_psum,
            tc.tile_pool(name="o_psum", bufs=2, space="PSUM") as o_psum,
            tc.tile_pool(name="dram", bufs=1, space="DRAM") as dram,
        ):
            ones8 = const.tile([P, 8, 1], BF16)
            nc.vector.memset(ones8, 1.0)
            bo_bc = persist.tile([P, DIM], F32)
            nc.sync.dma_start(bo_bc, bo[:].to_broadcast((P, DIM)))

            # SBUF-resident operands, [c, ck, free] with c = ck*128 + p
            xq_sb = persist.tile([P, 8, N], BF16, name="xq_sb")
            xk_sb = persist.tile([P, 8, S], BF16, name="xk_sb")
            xv_sb = persist.tile([P, 8, S], BF16, name="xv_sb")
            wk_sb = persist.tile([P, 8, H * DH], BF16, name="wk_sb")
            wv_sb = persist.tile([P, 8, H * DH], BF16, name="wv_sb")
            # wq's slot is reused by wo (wq is dead once q js=1 is emitted)
            wq_sb = wtail.tile([P, 8, H * DH], BF16, tag="wt", name="wq_sb")

            def load_in(src, dst):
                v = src[:].rearrange("(ck c) f -> c ck f", c=P)
                for i in range(4):
                    nc.sync.dma_start(dst[:, 2 * i : 2 * i + 2, :], v[:, 2 * i : 2 * i + 2, :])

            # load order: q-projection operands first so stage A starts early
            load_in(wqT, wq_sb)
            load_in(xqT, xq_sb)
            load_in(wkT, wk_sb)
            load_in(xkT, xk_sb)
            load_in(wvT, wv_sb)
            load_in(xvT, xv_sb)

            # projections, SBUF-resident: QnT/KnT [j = t*128+p, n] split by n
            # half; Vn [n = t*128+p, j] split by n half (t 0-3 / 4-7)
            QnT_h = [persist.tile([P, 8, 512], BF16, name=f"QnT_{i}") for i in range(2)]
            KnT_h = [persist.tile([P, 8, 512], BF16, name=f"KnT_{i}") for i in range(2)]
            Vn_h = [persist.tile([P, 4, 1024], BF16, name=f"Vn_{i}") for i in range(2)]
            outT = persist.tile([P, 8, N], BF16, name="outT")

            def gemm_group(lhs_sb, rhs_sb, t, js, dst_ap, tag):
                """one output tile [128, 512]: 8 accumulating MMs + cast"""
                pp = mm_psum.tile([P, 512], F32, tag="mm", name=f"pp_{tag}_{t}_{js}")
                for ck in range(8):
                    nc.tensor.matmul(
                        pp,
                        lhs_sb[:, ck, t * P : (t + 1) * P],
                        rhs_sb[:, ck, js * 512 : (js + 1) * 512],
                        start=(ck == 0),
                        stop=(ck == 7),
                    )
                nc.vector.tensor_copy(dst_ap, pp)

            def q_group(t, js):
                gemm_group(wq_sb, xq_sb, t, js, QnT_h[js][:, t, :], "q")

            def k_group(t, js):
                gemm_group(wk_sb, xk_sb, t, js, KnT_h[js][:, t, :], "k")

            def v_group(t, js):
                gemm_group(xv_sb, wv_sb, t, js, Vn_h[t // 4][:, t % 4, js * 512 : (js + 1) * 512], "v")

            # ---- stage A first half: everything heads 0-7 need ----
            for t in range(8):
                q_group(t, 0)
            for t in range(8):
                k_group(t, 0)
            for t in range(4):
                v_group(t, 0)
                v_group(t, 1)

            # remaining projection groups, woven into the first head pairs
            weave = (
                [("q", t, 1) for t in range(8)]
                + [("k", t, 1) for t in range(8)]
                + [("v", t, js) for t in range(4, 8) for js in range(2)]
            )
            weave_fns = {"q": q_group, "k": k_group, "v": v_group}
            weave_i = 0

            def weave_one():
                nonlocal weave_i
                if weave_i < len(weave):
                    kind, t, js = weave[weave_i]
                    weave_fns[kind](t, js)
                    weave_i += 1

            # wo loads into wq's slot; emitted after the weave list is built
            # but its DMA only runs once q js=1 has consumed wq.
            def emit_wo_load():
                wo_sb = wtail.tile([P, 8, DIM], BF16, tag="wt", name="wo_sb")
                load_in(woT, wo_sb)
                return wo_sb

            # ---- head pairs ----
            def gathers(u):
                """prefetchable head-pair operand gathers (SBUF->SBUF DMA)"""
                h0, h1 = 2 * u, 2 * u + 1
                half = u // 4  # which n-half of QnT/KnT (heads 0-7 vs 8-15)
                c0 = h0 * 64 - half * 512
                c1 = h1 * 64 - half * 512

                # kT2/qT2: [128, 1024] bf16; partitions 0-63 = head h0's
                # [d, n~], 64-127 = head h1's. n~ = g*64+r (permuted order).
                kT2 = heads.tile([P, 1024], BF16, tag="kT2", name=f"kT2_{u}")
                qT2 = heads.tile([P, 1024], BF16, tag="qT2", name=f"qT2_{u}")
                for dst, src_t, cc in (
                    (kT2[0:64, :], KnT_h[half], c0),
                    (kT2[64:128, :], KnT_h[half], c1),
                    (qT2[0:64, :], QnT_h[half], c0),
                    (qT2[64:128, :], QnT_h[half], c1),
                ):
                    # dst logical [d, n~ = (2t+gp)*64 + r]; one DMA per gp
                    dv = dst.rearrange("d (t gp r) -> d t gp r", gp=2, r=64)
                    for gp in range(2):
                        nc.sync.dma_start(
                            dv[:, :, gp, :],
                            src_t[gp * 64 : (gp + 1) * 64, :, cc : cc + 64],
                        )

                # v66[p, sk, 0:64] = v~[sk*128+p, d]; col 64 = ones (rowsum)
                def v_gather(h):
                    v66 = v66p.tile([P, 8, 65], BF16, tag="v66", name=f"v66_{h}")
                    t = h // 2
                    r0 = (h % 2) * 64
                    src = Vn_h[t // 4][r0 : r0 + 64, t % 4, :].rearrange(
                        "r (s gp d) -> r s gp d", gp=2, d=DH
                    )
                    for gp in range(2):
                        nc.sync.dma_start(
                            v66[gp * 64 : (gp + 1) * 64, :, 0:64], src[:, :, gp, :]
                        )
                    nc.vector.tensor_copy(v66[:, :, 64:65], ones8)
                    return v66

                return kT2, qT2, v_gather(h0), v_gather(h1)

            pref = {0: gathers(0)}
            chunk_no = [0]

            def pair(u):
                h0, h1 = 2 * u, 2 * u + 1
                kT2, qT2, v66_0, v66_1 = pref.pop(u)

                po_0 = o_psum.tile([P, 1024], F32, tag="po", name=f"po_{h0}")
                po_1 = o_psum.tile([P, 1024], F32, tag="po", name=f"po_{h1}")

                for sk in range(8):
                    ps_0 = mm_psum.tile([P, 1024], F32, tag="mm", name=f"ps_{h0}_{sk}")
                    ps_1 = mm_psum.tile([P, 1024], F32, tag="mm", name=f"ps_{h1}_{sk}")
                    # row-packed scores: head h0 on array rows 0-63, h1 on 64-127
                    for ns in range(2):
                        nc.tensor.matmul(
                            ps_0[:, ns * 512 : (ns + 1) * 512],
                            kT2[0:64, P * sk : P * (sk + 1)],
                            qT2[0:64, ns * 512 : (ns + 1) * 512],
                            start=True,
                            stop=True,
                        )
                        nc.tensor.matmul(
                            ps_1[:, ns * 512 : (ns + 1) * 512],
                            kT2[64:128, P * sk : P * (sk + 1)],
                            qT2[64:128, ns * 512 : (ns + 1) * 512],
                            start=True,
                            stop=True,
                        )
                    pexp_0 = work.tile([P, 1024], BF16, tag="pexp", name="pexp0")
                    nc.scalar.activation(pexp_0, ps_0, AF.Exp, scale=SCALE)
                    pexp_1 = work.tile([P, 1024], BF16, tag="pexp", name="pexp1")
                    nc.scalar.activation(pexp_1, ps_1, AF.Exp, scale=SCALE)
                    for po, v66, pexp in ((po_0, v66_0, pexp_0), (po_1, v66_1, pexp_1)):
                        for ns in range(2):
                            nc.tensor.matmul(
                                po[0:65, ns * 512 : (ns + 1) * 512],
                                v66[:, sk, 0:65],
                                pexp[:, ns * 512 : (ns + 1) * 512],
                                start=(sk == 0),
                                stop=(sk == 7),
                            )
                    # one projection group per 2 chunks keeps PE ~= ACT pace
                    if chunk_no[0] % 2 == 0:
                        weave_one()
                    chunk_no[0] += 1
                    # prefetch next pair's gathers ahead of this pair's
                    # divide-chain DMAs (avoids DMA-queue head-of-line block)
                    if sk == 2 and u + 1 < 8:
                        pref[u + 1] = gathers(u + 1)

                # softmax divide; Z on psum partition 64. Everything stays in
                # n~ order (outT holds outT-tilde); the final stage-C output
                # DMA applies n~ -> n' for free. The Z row is spread across
                # 128 partitions (SBUF->SBUF DMA) so the DVE reciprocal runs
                # 128 lanes wide, then DRAM-bounce broadcast.
                jk = u
                for hi, po in ((0, po_0), (1, po_1)):
                    zsb = sm.tile([65, 1024], BF16, tag="zsb", name=f"zsb_{u}_{hi}")
                    with nc.allow_low_precision(reason="softmax denom in bf16"):
                        nc.vector.tensor_copy(zsb[64:65, :], po[64:65, :])
                    zrow = dram.tile([1, 1024], BF16, tag="zrow", name=f"zrow_{u}_{hi}")
                    nc.sync.dma_start(zrow, zsb[64:65, :])
                    zsp = sm.tile([P, 8], BF16, tag="zsp", name=f"zsp_{u}_{hi}")
                    nc.sync.dma_start(zsp, zrow[:].rearrange("o (p e) -> (o p) e", p=P))
                    rsp = sm.tile([P, 8], BF16, tag="rsp", name=f"rsp_{u}_{hi}")
                    with nc.allow_low_precision(reason="softmax denom recip in bf16"):
                        nc.vector.reciprocal(rsp, zsp)
                    rrow = dram.tile([1, 1024], BF16, tag="rrow", name=f"rrow_{u}_{hi}")
                    nc.sync.dma_start(rrow[:].rearrange("o (p e) -> (o p) e", p=P), rsp)
                    rbc = sm.tile([64, 1024], BF16, tag="rbc", name=f"rbc_{u}_{hi}")
                    nc.sync.dma_start(rbc, rrow[:].to_broadcast((64, 1024)))
                    if hi == 0:
                        nc.vector.tensor_mul(out=outT[0:64, jk, :], in0=po[0:64, :], in1=rbc)
                    else:
                        tmpo = sm.tile([64, 1024], BF16, tag="tmpo", name=f"tmpo_{u}")
                        nc.vector.tensor_mul(out=tmpo, in0=po[0:64, :], in1=rbc)
                        nc.sync.dma_start(outT[64:128, jk, :], tmpo)

            for u in range(4):
                pair(u)
            wo_sb = emit_wo_load()
            for u in range(4, 8):
                pair(u)

            # ---- stage C: out = outT.T @ woT + bo ----
            for m in range(8):
                for isl in range(2):
                    pf = mm_psum.tile([P, 512], F32, tag="mm", name=f"pf_{m}_{isl}")
                    for ck in range(8):
                        nc.tensor.matmul(
                            pf,
                            outT[:, ck, m * P : (m + 1) * P],
                            wo_sb[:, ck, isl * 512 : (isl + 1) * 512],
                            start=(ck == 0),
                            stop=(ck == 7),
                        )
                    fin = sm.tile([P, 512], F32, tag="fin", name=f"fin_{m}_{isl}")
                    nc.vector.tensor_add(out=fin, in0=pf, in1=bo_bc[:, isl * 512 : (isl + 1) * 512])
                    # out rows stay in n~ order; the host applies n~ -> n'
                    nc.sync.dma_start(out[m * P : (m + 1) * P, isl * 512 : (isl + 1) * 512], fin)

    nc.compile()
    return nc


_NC_CACHE = {}


def _get_nc():
    if "nc" not in _NC_CACHE:
        _NC_CACHE["nc"] = build()
    return _NC_CACHE["nc"]


TRACE = False


def kernel(queries, keys, values, Wq, Wk, Wv, Wo, bo):
    import ml_dtypes
    from concourse.bass_utils import run_bass_kernel_spmd

    bf16 = ml_dtypes.bfloat16
    qT = [np.ascontiguousarray(np.asarray(queries[i], np.float32).T).astype(bf16) for i in range(B)]
    kT = [np.ascontiguousarray(np.asarray(keys[i], np.float32).T).astype(bf16) for i in range(B)]
    vT = [np.ascontiguousarray(np.asarray(values[i], np.float32).T).astype(bf16) for i in range(B)]
    wqT = np.ascontiguousarray(np.asarray(Wq, np.float32).T).astype(bf16)
    wkT = np.ascontiguousarray(np.asarray(Wk, np.float32).T).astype(bf16)
    wvT = np.ascontiguousarray(np.asarray(Wv, np.float32).T).astype(bf16)
    woT = np.ascontiguousarray(np.asarray(Wo, np.float32).T).astype(bf16)
    bo2 = np.ascontiguousarray(np.asarray(bo, np.float32).reshape(1, DIM))

    nc = _get_nc()
    in_maps = [
        {
            "xqT": qT[i],
            "xkT": kT[i],
            "xvT": vT[i],
            "wqT": wqT,
            "wkT": wkT,
            "wvT": wvT,
            "woT": woT,
            "bo": bo2,
        }
        for i in range(B)
    ]
    res = run_bass_kernel_spmd(nc, in_maps, core_ids=list(range(B)), trace=TRACE)
    if TRACE:
        _NC_CACHE["last_results"] = res
    # device rows are in n~ = g*64+r order; O-row n' = r*16+g
    ar = np.arange(N)
    idx = (ar % 16) * 64 + ar // 16  # n~ holding row n'
    return np.stack([res.results[i]["out"][idx] for i in range(B)])


# revision 14
# speedup vs baseline: 1.6223x; 1.2416x over previous
"""CrossAttention Trainium2 Bass kernel (v3: SBUF-resident intermediates,
row-packed scores, permuted head ordering, interleaved projections).

Problem (hardcoded shapes): B=8, N=S=1024, DIM=1024, H=16, DH=64.
  q = (queries @ Wq.T).reshape(B, H, N, DH)   # direct reshape, NOT a head transpose
  attn = softmax(q @ k^T * DH**-0.5); out = attn @ v
  out = out.transpose(0,2,1,3).reshape(B,N,H*DH) @ Wo.T + bo
Sharding: data-parallel over batch B (one batch element per core, weights
replicated, no collectives).

Key ideas vs v2 (391us baseline):
- QnT/KnT/Vn stay SBUF-resident (split into js/t halves as separate tiles so
  heads 0-7 only depend on the first half of the projections); head operand
  gathers are SBUF->SBUF DMA, no DRAM bounce.
- Heads use the permuted local ordering n~ = g*64+r (instead of the torch
  n' = r*16+g): the gather then lands directly in matmul layout (no DVE
  reorder); softmax is permutation-equivariant, and the inverse permutation
  is applied for free in the divide's strided write to outT.
- Scores for a head pair run as concurrent 64-row PE tiles ((0,0)/(64,0)):
  contraction is DH=64, so two heads share the 128x128 array -> 2x scores.
- The second half of the projections (q/k js=1, v t=4..7) is woven between
  the first 4 head-pairs' chunks, sharing one PSUM tag, so the PE stays
  dense (HAM warm) while ScalarE exp is the per-pair bottleneck.
- Softmax divide: DVE reciprocal straight from the PSUM Z row, one broadcast
  DMA, one tensor_mul (strided dst applies the n~ -> n' permutation).
"""

import numpy as np

import concourse.bass as bass
import concourse.mybir as mybir
import concourse.tile as tile
from concourse import bacc

B, N, S, DIM, H, DH = 8, 1024, 1024, 1024, 16, 64
SCALE = DH**-0.5
P = 128
F32 = mybir.dt.float32
BF16 = mybir.dt.bfloat16
AF = mybir.ActivationFunctionType


def build(debug: bool = False) -> bacc.Bacc:
    nc = bacc.Bacc("TRN2", target_bir_lowering=False, debug=debug, num_devices=B)

    xqT = nc.dram_tensor("xqT", [DIM, N], BF16, kind="ExternalInput")
    xkT = nc.dram_tensor("xkT", [DIM, S], BF16, kind="ExternalInput")
    xvT = nc.dram_tensor("xvT", [DIM, S], BF16, kind="ExternalInput")
    wqT = nc.dram_tensor("wqT", [DIM, H * DH], BF16, kind="ExternalInput")
    wkT = nc.dram_tensor("wkT", [DIM, H * DH], BF16, kind="ExternalInput")
    wvT = nc.dram_tensor("wvT", [DIM, H * DH], BF16, kind="ExternalInput")
    woT = nc.dram_tensor("woT", [H * DH, DIM], BF16, kind="ExternalInput")
    bo = nc.dram_tensor("bo", [1, DIM], F32, kind="ExternalInput")
    out = nc.dram_tensor("out", [N, DIM], F32, kind="ExternalOutput")

    with tile.TileContext(nc) as tc:
        with (
            tc.tile_pool(name="const", bufs=1) as const,
            tc.tile_pool(name="persist", bufs=1) as persist,
            tc.tile_pool(name="wtail", bufs=1) as wtail,
            tc.tile_pool(name="heads", bufs=2) as heads,
            tc.tile_pool(name="v66p", bufs=4) as v66p,
            tc.tile_pool(name="work", bufs=3) as work,
            tc.tile_pool(name="sm", bufs=2) as sm,
            tc.tile_pool(name="mm_psum", bufs=2, space="PSUM") as mm_psum,
            tc.tile_pool(name="o_psum", bufs=2, space="PSUM") as o_psum,
            tc.tile_pool(name="dram", bufs=1, space="DRAM") as dram,
        ):
            ones8 = const.tile([P, 8, 1], BF16)
            nc.vector.memset(ones8, 1.0)
            bo_bc = persist.tile([P, DIM], F32)
            nc.sync.dma_start(bo_bc, bo[:].to_broadcast((P, DIM)))

            # SBUF-resident operands, [c, ck, free] with c = ck*128 + p
            xq_sb = persist.tile([P, 8, N], BF16, name="xq_sb")
            xk_sb = persist.tile([P, 8, S], BF16, name="xk_sb")
            xv_sb = persist.tile([P, 8, S], BF16, name="xv_sb")
            wk_sb = persist.tile([P, 8, H * DH], BF16, name="wk_sb")
            wv_sb = persist.tile([P, 8, H * DH], BF16, name="wv_sb")
            # wq's slot is reused by wo (wq is dead once q js=1 is emitted)
            wq_sb = wtail.tile([P, 8, H * DH], BF16, tag="wt", name="wq_sb")

            def load_in(src, dst):
                v = src[:].rearrange("(ck c) f -> c ck f", c=P)
                for i in range(4):
                    nc.sync.dma_start(dst[:, 2 * i : 2 * i + 2, :], v[:, 2 * i : 2 * i + 2, :])

            # load order: q-projection operands first so stage A starts early
            load_in(wqT, wq_sb)
            load_in(xqT, xq_sb)
            load_in(wkT, wk_sb)
            load_in(xkT, xk_sb)
            load_in(wvT, wv_sb)
            load_in(xvT, xv_sb)

            # projections, SBUF-resident: QnT/KnT [j = t*128+p, n] split by n
            # half; Vn [n = t*128+p, j] split by n half (t 0-3 / 4-7)
            QnT_h = [persist.tile([P, 8, 512], BF16, name=f"QnT_{i}") for i in range(2)]
            KnT_h = [persist.tile([P, 8, 512], BF16, name=f"KnT_{i}") for i in range(2)]
            Vn_h = [persist.tile([P, 4, 1024], BF16, name=f"Vn_{i}") for i in range(2)]
            outT = persist.tile([P, 8, N], BF16, name="outT")

            def gemm_group(lhs_sb, rhs_sb, t, js, dst_ap, tag):
                """one output tile [128, 512]: 8 accumulating MMs + cast"""
                pp = mm_psum.tile([P, 512], F32, tag="mm", name=f"pp_{tag}_{t}_{js}")
                for ck in range(8):
                    nc.tensor.matmul(
                        pp,
                        lhs_sb[:, ck, t * P : (t + 1) * P],
                        rhs_sb[:, ck, js * 512 : (js + 1) * 512],
                        start=(ck == 0),
                        stop=(ck == 7),
                    )
                nc.vector.tensor_copy(dst_ap, pp)

            def q_group(t, js):
                gemm_group(wq_sb, xq_sb, t, js, QnT_h[js][:, t, :], "q")

            def k_group(t, js):
                gemm_group(wk_sb, xk_sb, t, js, KnT_h[js][:, t, :], "k")

            def v_group(t, js):
                gemm_group(xv_sb, wv_sb, t, js, Vn_h[t // 4][:, t % 4, js * 512 : (js + 1) * 512], "v")

            # ---- stage A first half: everything heads 0-7 need ----
            for t in range(8):
                q_group(t, 0)
            for t in range(8):
                k_group(t, 0)
            for t in range(4):
                v_group(t, 0)
                v_group(t, 1)

            # remaining projection groups, woven into the first head pairs
            weave = (
                [("q", t, 1) for t in range(8)]
                + [("k", t, 1) for t in range(8)]
                + [("v", t, js) for t in range(4, 8) for js in range(2)]
            )
            weave_fns = {"q": q_group, "k": k_group, "v": v_group}
            weave_i = 0

            def weave_one():
                nonlocal weave_i
                if weave_i < len(weave):
                    kind, t, js = weave[weave_i]
                    weave_fns[kind](t, js)
                    weave_i += 1

            # wo loads into wq's slot; emitted after the weave list is built
            # but its DMA only runs once q js=1 has consumed wq.
            def emit_wo_load():
                wo_sb = wtail.tile([P, 8, DIM], BF16, tag="wt", name="wo_sb")
                load_in(woT, wo_sb)
                return wo_sb

            # ---- head pairs ----
            def gathers(u):
                """prefetchable head-pair operand gathers (SBUF->SBUF DMA)"""
                h0, h1 = 2 * u, 2 * u + 1
                half = u // 4  # which n-half of QnT/KnT (heads 0-7 vs 8-15)
                c0 = h0 * 64 - half * 512
                c1 = h1 * 64 - half * 512

                # kT2/qT2: [128, 1024] bf16; partitions 0-63 = head h0's
                # [d, n~], 64-127 = head h1's. n~ = g*64+r (permuted order).
                kT2 = heads.tile([P, 1024], BF16, tag="kT2", name=f"kT2_{u}")
                qT2 = heads.tile([P, 1024], BF16, tag="qT2", name=f"qT2_{u}")
                for dst, src_t, cc in (
                    (kT2[0:64, :], KnT_h[half], c0),
                    (kT2[64:128, :], KnT_h[half], c1),
                    (qT2[0:64, :], QnT_h[half], c0),
                    (qT2[64:128, :], QnT_h[half], c1),
                ):
                    # dst logical [d, n~ = (2t+gp)*64 + r]; one DMA per gp
                    dv = dst.rearrange("d (t gp r) -> d t gp r", gp=2, r=64)
                    for gp in range(2):
                        nc.sync.dma_start(
                            dv[:, :, gp, :],
                            src_t[gp * 64 : (gp + 1) * 64, :, cc : cc + 64],
                        )

                # v66[p, sk, 0:64] = v~[sk*128+p, d]; col 64 = ones (rowsum)
                def v_gather(h):
                    v66 = v66p.tile([P, 8, 65], BF16, tag="v66", name=f"v66_{h}")
                    t = h // 2
                    r0 = (h % 2) * 64
                    src = Vn_h[t // 4][r0 : r0 + 64, t % 4, :].rearrange(
                        "r (s gp d) -> r s gp d", gp=2, d=DH
                    )
                    for gp in range(2):
                        nc.sync.dma_start(
                            v66[gp * 64 : (gp + 1) * 64, :, 0:64], src[:, :, gp, :]
                        )
                    nc.vector.tensor_copy(v66[:, :, 64:65], ones8)
                    return v66

                return kT2, qT2, v_gather(h0), v_gather(h1)

            pref = {0: gathers(0)}
            chunk_no = [0]

            def pair(u):
                h0, h1 = 2 * u, 2 * u + 1
                kT2, qT2, v66_0, v66_1 = pref.pop(u)

                po_0 = o_psum.tile([P, 1024], F32, tag="po", name=f"po_{h0}")
                po_1 = o_psum.tile([P, 1024], F32, tag="po", name=f"po_{h1}")

                for sk in range(8):
                    ps_0 = mm_psum.tile([P, 1024], F32, tag="mm", name=f"ps_{h0}_{sk}")
                    ps_1 = mm_psum.tile([P, 1024], F32, tag="mm", name=f"ps_{h1}_{sk}")
                    # row-packed scores: head h0 on array rows 0-63, h1 on 64-127
                    for ns in range(2):
                        nc.tensor.matmul(
                            ps_0[:, ns * 512 : (ns + 1) * 512],
                            kT2[0:64, P * sk : P * (sk + 1)],
                            qT2[0:64, ns * 512 : (ns + 1) * 512],
                            start=True,
                            stop=True,
                        )
                        nc.tensor.matmul(
                            ps_1[:, ns * 512 : (ns + 1) * 512],
                            kT2[64:128, P * sk : P * (sk + 1)],
                            qT2[64:128, ns * 512 : (ns + 1) * 512],
                            start=True,
                            stop=True,
                        )
                    pexp_0 = work.tile([P, 1024], BF16, tag="pexp", name="pexp0")
                    nc.scalar.activation(pexp_0, ps_0, AF.Exp, scale=SCALE)
                    pexp_1 = work.tile([P, 1024], BF16, tag="pexp", name="pexp1")
                    nc.scalar.activation(pexp_1, ps_1, AF.Exp, scale=SCALE)
                    for po, v66, pexp in ((po_0, v66_0, pexp_0), (po_1, v66_1, pexp_1)):
                        for ns in range(2):
                            nc.tensor.matmul(
                                po[0:65, ns * 512 : (ns + 1) * 512],
                                v66[:, sk, 0:65],
                                pexp[:, ns * 512 : (ns + 1) * 512],
                                start=(sk == 0),
                                stop=(sk == 7),
                            )
                    # one projection group per 2 chunks keeps PE ~= ACT pace
                    if chunk_no[0] % 2 == 0:
                        weave_one()
                    chunk_no[0] += 1
                    # prefetch next pair's gathers ahead of this pair's
                    # divide-chain DMAs (avoids DMA-queue head-of-line block)
                    if sk == 2 and u + 1 < 8:
                        pref[u + 1] = gathers(u + 1)

                # softmax divide; Z on psum partition 64. Everything stays in
                # n~ order (outT holds outT-tilde); the final stage-C output
                # DMA applies n~ -> n' for free. The Z row is spread across
                # 128 partitions (SBUF->SBUF DMA) so the DVE reciprocal runs
                # 128 lanes wide, then DRAM-bounce broadcast.
                jk = u
                for hi, po in ((0, po_0), (1, po_1)):
                    # single copy evacuates unnormalized out + Z row, freeing
                    # the po PSUM banks ~1us after the last AV; the whole
                    # reciprocal/broadcast/mul chain then runs off the
                    # PE-critical path.
                    stg65 = sm.tile([65, 1024], BF16, tag="stg65", name=f"stg65_{u}_{hi}")
                    with nc.allow_low_precision(reason="unnormalized attn out in bf16"):
                        nc.vector.tensor_copy(stg65, po[0:65, :])
                    zrow = dram.tile([1, 1024], BF16, tag="zrow", name=f"zrow_{u}_{hi}")
                    nc.sync.dma_start(zrow, stg65[64:65, :])
                    zsp = sm.tile([P, 8], BF16, tag="zsp", name=f"zsp_{u}_{hi}")
                    nc.sync.dma_start(zsp, zrow[:].rearrange("o (p e) -> (o p) e", p=P))
                    rsp = sm.tile([P, 8], BF16, tag="rsp", name=f"rsp_{u}_{hi}")
                    with nc.allow_low_precision(reason="softmax denom recip in bf16"):
                        nc.vector.reciprocal(rsp, zsp)
                    rrow = dram.tile([1, 1024], BF16, tag="rrow", name=f"rrow_{u}_{hi}")
                    nc.sync.dma_start(rrow[:].rearrange("o (p e) -> (o p) e", p=P), rsp)
                    rbc = sm.tile([64, 1024], BF16, tag="rbc", name=f"rbc_{u}_{hi}")
                    nc.sync.dma_start(rbc, rrow[:].to_broadcast((64, 1024)))
                    if hi == 0:
                        nc.vector.tensor_mul(out=outT[0:64, jk, :], in0=stg65[0:64, :], in1=rbc)
                    else:
                        tmpo = sm.tile([64, 1024], BF16, tag="tmpo", name=f"tmpo_{u}")
                        nc.vector.tensor_mul(out=tmpo, in0=stg65[0:64, :], in1=rbc)
                        nc.sync.dma_start(outT[64:128, jk, :], tmpo)

            for u in range(4):
                pair(u)
            wo_sb = emit_wo_load()
            for u in range(4, 8):
                pair(u)

            # ---- stage C: out = outT.T @ woT + bo ----
            for m in range(8):
                for isl in range(2):
                    pf = mm_psum.tile([P, 512], F32, tag="mm", name=f"pf_{m}_{isl}")
                    for ck in range(8):
                        nc.tensor.matmul(
                            pf,
                            outT[:, ck, m * P : (m + 1) * P],
                            wo_sb[:, ck, isl * 512 : (isl + 1) * 512],
                            start=(ck == 0),
                            stop=(ck == 7),
                        )
                    fin = sm.tile([P, 512], F32, tag="fin", name=f"fin_{m}_{isl}")
                    nc.vector.tensor_add(out=fin, in0=pf, in1=bo_bc[:, isl * 512 : (isl + 1) * 512])
                    # out rows stay in n~ order; the host applies n~ -> n'
                    nc.sync.dma_start(out[m * P : (m + 1) * P, isl * 512 : (isl + 1) * 512], fin)

    nc.compile()
    return nc


_NC_CACHE = {}


def _get_nc():
    if "nc" not in _NC_CACHE:
        _NC_CACHE["nc"] = build()
    return _NC_CACHE["nc"]


TRACE = False


def kernel(queries, keys, values, Wq, Wk, Wv, Wo, bo):
    import ml_dtypes
    from concourse.bass_utils import run_bass_kernel_spmd

    bf16 = ml_dtypes.bfloat16
    qT = [np.ascontiguousarray(np.asarray(queries[i], np.float32).T).astype(bf16) for i in range(B)]
    kT = [np.ascontiguousarray(np.asarray(keys[i], np.float32).T).astype(bf16) for i in range(B)]
    vT = [np.ascontiguousarray(np.asarray(values[i], np.float32).T).astype(bf16) for i in range(B)]
    wqT = np.ascontiguousarray(np.asarray(Wq, np.float32).T).astype(bf16)
    wkT = np.ascontiguousarray(np.asarray(Wk, np.float32).T).astype(bf16)
    wvT = np.ascontiguousarray(np.asarray(Wv, np.float32).T).astype(bf16)
    woT = np.ascontiguousarray(np.asarray(Wo, np.float32).T).astype(bf16)
    bo2 = np.ascontiguousarray(np.asarray(bo, np.float32).reshape(1, DIM))

    nc = _get_nc()
    in_maps = [
        {
            "xqT": qT[i],
            "xkT": kT[i],
            "xvT": vT[i],
            "wqT": wqT,
            "wkT": wkT,
            "wvT": wvT,
            "woT": woT,
            "bo": bo2,
        }
        for i in range(B)
    ]
    res = run_bass_kernel_spmd(nc, in_maps, core_ids=list(range(B)), trace=TRACE)
    if TRACE:
        _NC_CACHE["last_results"] = res
    # device rows are in n~ = g*64+r order; O-row n' = r*16+g
    ar = np.arange(N)
    idx = (ar % 16) * 64 + ar // 16  # n~ holding row n'
    return np.stack([res.results[i]["out"][idx] for i in range(B)])
